# revision 1
# baseline (speedup 1.0000x reference)
"""Bass/Trainium2 kernel for nn_Head_13030930776875.

out = 0.7*softmax(causal(x@Wq @ (x@Wk)^T / sqrt(d))) @ (x@Wv)
    + 0.3*rownorm(causal(exp(-|y_i - y_j|^2 / (2d)))) @ (x@Wv),  y = (x@Wk)@L_grav

Sharding: 8 cores = 4 samples x 2 halves. Each half owns two 512-row query
groups chosen so causal (triangular) work balances: half0 -> {G0, G3},
half1 -> {G1, G2}. The program is SPMD-uniform; per-half differences are
carried in input data (masks, exp-bias gates of -1e30).

On-device layout: everything transposed (d on partitions). Scores are
computed as s^T tiles [k,q] so that (a) A^T slices feed the A@v matmul
directly as the stationary operand (no transposes anywhere), and (b) the
causal row-sums come free via a ones-column appended to v. The grav kernel
exp(-dist2/2d) is factorized; the per-query factor cancels in the row
normalization and the per-key factor -sq_k/256 rides the ACT exp bias.
"""

import math
import os

import numpy as np

B, N, D_MODEL, D_HEAD = 4, 2048, 1024, 128
OMEGA_LANG, OMEGA_GRAV = 0.7, 0.3
SC_LANG = 1.0 / math.sqrt(D_HEAD)
SC_GRAV = 1.0 / D_HEAD
NEG = -1.0e30
NBLK = N // 128            # 16 k-chunks of 128
NCH = (8, 16)              # chunks per position (pos0 group, pos1 group)
NSLOT = NCH[0] + NCH[1]    # 24 mask/bias slots

_CACHE = {}


def _build_nc():
    import concourse.bacc as bacc
    import concourse.mybir as mybir
    import concourse.tile as tile
    import concourse.bass as bass

    dt = mybir.dt
    F16, F32 = dt.float16, dt.float32
    AF = mybir.ActivationFunctionType
    OP = mybir.AluOpType

    nc = bacc.Bacc()

    xT = nc.declare_dram_parameter("xT", [D_MODEL, N], F16, isOutput=False)
    xqT = nc.declare_dram_parameter("xqT", [D_MODEL, N // 2], F16, isOutput=False)
    wq = nc.declare_dram_parameter("wq", [128, 8 * 128], F16, isOutput=False)
    wk = nc.declare_dram_parameter("wk", [128, 8 * 128], F16, isOutput=False)
    wv = nc.declare_dram_parameter("wv", [128, 8 * 128], F16, isOutput=False)
    wy = nc.declare_dram_parameter("wy", [128, 8 * 128], F16, isOutput=False)
    lg = nc.declare_dram_parameter("lg", [128, 128], F16, isOutput=False)
    iota = nc.declare_dram_parameter("iota", [128, 512], F16, isOutput=False)
    thr = nc.declare_dram_parameter("thr", [128, NSLOT], F32, isOutput=False)
    lbias = nc.declare_dram_parameter("lbias", [128, NSLOT], F32, isOutput=False)
    gbias = nc.declare_dram_parameter("gbias", [128, NSLOT], F32, isOutput=False)
    out_d = nc.declare_dram_parameter("out", [N // 2, 128], F32, isOutput=True)

    with tile.TileContext(nc) as tc:
        with (
            tc.tile_pool(name="big", bufs=1) as big,
            tc.tile_pool(name="xtp", bufs=8) as xtp,
            tc.tile_pool(name="xqp", bufs=8) as xqp,
            tc.tile_pool(name="ap", bufs=2) as apool,
            tc.tile_pool(name="small", bufs=4) as small,
            tc.tile_pool(name="outp", bufs=4) as outp,
            tc.tile_pool(name="score", bufs=4, space="PSUM") as score,
            tc.tile_pool(name="pp", bufs=4, space="PSUM") as pp,
        ):
            # ---- small inputs first (weights gate every matmul) ----
            wq_s = big.tile([128, 8, 128], F16, tag="wq")
            wk_s = big.tile([128, 8, 128], F16, tag="wk")
            wv_s = big.tile([128, 8, 128], F16, tag="wv")
            wy_s = big.tile([128, 8, 128], F16, tag="wy")
            for t, d in ((wq_s, wq), (wk_s, wk), (wv_s, wv), (wy_s, wy)):
                nc.sync.dma_start(t[:], d[:].rearrange("p (c d) -> p c d", c=8))
            lg_s = big.tile([128, 128], F16, tag="lg")
            nc.sync.dma_start(lg_s[:], lg[:])
            io_s = big.tile([128, 512], F16, tag="io")
            nc.sync.dma_start(io_s[:], iota[:])
            th_s = big.tile([128, NSLOT], F32, tag="th")
            nc.sync.dma_start(th_s[:], thr[:])
            lb_s = big.tile([128, NSLOT], F32, tag="lb")
            nc.sync.dma_start(lb_s[:], lbias[:])
            gb_s = big.tile([128, NSLOT], F32, tag="gb")
            nc.sync.dma_start(gb_s[:], gbias[:])

            # ---- x^T / xq^T split by 512-column groups, group-major order ----
            xt = [xtp.tile([128, N], F16, tag="xt", name=f"xt{c}")
                  for c in range(8)]
            xqt = [xqp.tile([128, N // 2], F16, tag="xq", name=f"xqt{c}")
                   for c in range(8)]
            for g in range(4):
                cols = slice(g * 512, (g + 1) * 512)
                for c in range(8):
                    nc.sync.dma_start(xt[c][:, cols], xT[c * 128:(c + 1) * 128, cols])
                if g < 2:
                    for c in range(8):
                        nc.sync.dma_start(xqt[c][:, cols],
                                          xqT[c * 128:(c + 1) * 128, cols])

            # ---- projections, emitted group-sliced so pos0 unblocks early ----
            kT = big.tile([128, N], F16, tag="kT")
            qT = big.tile([128, N // 2], F16, tag="qT")
            yqT = big.tile([128, N // 2], F16, tag="yqT")
            yT = big.tile([128, N], F16, tag="yT")
            sqn = big.tile([128, NBLK], F32, tag="sqn")
            gvb = big.tile([128, NSLOT], F32, tag="gvb")
            vaug = big.tile([128, NBLK, 132], F16, tag="vaug")

            def proj_group(dst, w_sb, src, g):
                cols = slice(g * 512, (g + 1) * 512)
                ps = pp.tile([128, 512], F32, tag="pp")
                for c in range(8):
                    nc.tensor.matmul(ps[:], w_sb[:, c, :], src[c][:, cols],
                                     start=(c == 0), stop=(c == 7))
                nc.vector.tensor_copy(dst[:, cols], ps[:])

            def yt_group(g):
                cols = slice(g * 512, (g + 1) * 512)
                ps = pp.tile([128, 512], F32, tag="pp")
                nc.tensor.matmul(ps[:], lg_s[:], kT[:, cols])
                nc.vector.tensor_copy(yT[:, cols], ps[:])

            def sqn_chunk(kb):
                ps = pp.tile([128, 512], F32, tag="pp")
                nc.tensor.matmul(ps[:, 0:128], kT[:, kb * 128:(kb + 1) * 128], lg_s[:])
                scr = small.tile([128, 128], F32, tag="scr")
                nc.scalar.activation(scr[:], ps[:, 0:128], AF.Square,
                                     scale=0.0625, accum_out=sqn[:, kb:kb + 1])

            def vaug_chunk(kb):
                ps = pp.tile([128, 512], F32, tag="pp")
                for c in range(8):
                    nc.tensor.matmul(ps[:, 0:128], xt[c][:, kb * 128:(kb + 1) * 128],
                                     wv_s[:, c, :], start=(c == 0), stop=(c == 7))
                nc.vector.tensor_copy(vaug[:, kb, 0:128], ps[:, 0:128])
                nc.vector.memset(vaug[:, kb, 128:129], 1.0)

            # PE warmup: ~3.5us of dummy matmuls on lg so the HAM clock-gate
            # opens (4/8 -> 8/8) before the real stream begins.
            warm = pp.tile([128, 512], F32, tag="pp")
            for i in range(28):
                nc.tensor.matmul(warm[:, 0:128], lg_s[:], lg_s[:],
                                 start=(i == 0), stop=(i == 27))

            # device-generated causal masks: mk[slot] = (iota >= thr[slot])
            mk_s = big.tile([128, NSLOT * 512], F16, tag="mk")
            for slot in range(NSLOT):
                nc.vector.tensor_scalar(mk_s[:, slot * 512:(slot + 1) * 512],
                                        io_s[:], th_s[:, slot:slot + 1], None,
                                        OP.is_ge)

            # pos0 prerequisites: kT g0-g1, qT/yqT g0, yT g0-g1, sqn/gvb 0..7
            proj_group(kT, wk_s, xt, 0)
            proj_group(kT, wk_s, xt, 1)
            proj_group(qT, wq_s, xqt, 0)
            yt_group(0)
            yt_group(1)
            proj_group(yqT, wy_s, xqt, 0)
            for kb in range(8):
                sqn_chunk(kb)
            nc.vector.tensor_tensor(gvb[:, 0:8], gb_s[:, 0:8], sqn[:, 0:8], OP.subtract)
            for kb in range(8):
                vaug_chunk(kb)

            # ---- attention, per position (two query groups of 512) ----
            for pos in range(2):
                if pos == 1:
                    proj_group(kT, wk_s, xt, 2)
                    proj_group(kT, wk_s, xt, 3)
                    proj_group(qT, wq_s, xqt, 1)
                    yt_group(2)
                    yt_group(3)
                    proj_group(yqT, wy_s, xqt, 1)
                    for kb in range(8, 16):
                        sqn_chunk(kb)
                    nc.vector.tensor_tensor(gvb[:, 8:24], gb_s[:, 8:24],
                                            sqn[:, 0:16], OP.subtract)
                    for kb in range(8, 16):
                        vaug_chunk(kb)
                nch = NCH[pos]
                slot0 = 0 if pos == 0 else NCH[0]
                qoff = pos * 512
                alang = apool.tile([128, NCH[1] * 512], F16, tag="alang")
                agrav = apool.tile([128, NCH[1] * 512], F16, tag="agrav")
                for kb in range(nch):
                    if pos == 0:
                        shrink = 0 if kb < 4 else (kb - 4) * 128
                    else:
                        shrink = 0 if kb < 12 else (kb - 12) * 128
                    w = 512 - shrink
                    slot = slot0 + kb
                    kcols = slice(kb * 128, (kb + 1) * 128)
                    acols = slice(kb * 512 + shrink, (kb + 1) * 512)
                    # lang: s^T = k_blk @ q^T
                    ps = score.tile([128, 512], F32, tag="sc")
                    nc.tensor.matmul(ps[:, 0:w], kT[:, kcols],
                                     qT[:, qoff + shrink:qoff + 512])
                    nc.scalar.activation(alang[:, acols], ps[:, 0:w], AF.Exp,
                                         bias=lb_s[:, slot:slot + 1], scale=SC_LANG)
                    # grav: gram^T = y_blk @ yq^T ; kern~ = exp(gram/128 - sq_k/256)
                    pg = score.tile([128, 512], F32, tag="sc")
                    nc.tensor.matmul(pg[:, 0:w], yT[:, kcols],
                                     yqT[:, qoff + shrink:qoff + 512])
                    nc.scalar.activation(agrav[:, acols], pg[:, 0:w], AF.Exp,
                                         bias=gvb[:, slot:slot + 1], scale=SC_GRAV)
                    # causal mask multiply (2x-mode TT against device-built mask)
                    if pos == 0 or kb >= 8:
                        mcols = slice(slot * 512 + shrink, (slot + 1) * 512)
                        nc.vector.tensor_tensor(alang[:, acols], alang[:, acols],
                                                mk_s[:, mcols], OP.mult)
                        nc.vector.tensor_tensor(agrav[:, acols], agrav[:, acols],
                                                mk_s[:, mcols], OP.mult)
                # A^T @ v_aug per 128-row query block
                for j in range(4):
                    nkb = (5 + j) if pos == 0 else (13 + j)
                    pol = pp.tile([128, 132], F32, tag="pp")
                    pog = pp.tile([128, 132], F32, tag="pp")
                    for kb in range(nkb):
                        nc.tensor.matmul(pol[:, 0:129],
                                         alang[:, kb * 512 + j * 128:kb * 512 + (j + 1) * 128],
                                         vaug[:, kb, 0:129],
                                         start=(kb == 0), stop=(kb == nkb - 1))
                    for kb in range(nkb):
                        nc.tensor.matmul(pog[:, 0:129],
                                         agrav[:, kb * 512 + j * 128:kb * 512 + (j + 1) * 128],
                                         vaug[:, kb, 0:129],
                                         start=(kb == 0), stop=(kb == nkb - 1))
                    rl = small.tile([128, 1], F32, tag="rl")
                    rg = small.tile([128, 1], F32, tag="rg")
                    nc.vector.reciprocal(rl[:], pol[:, 128:129])
                    nc.vector.tensor_scalar(rl[:], rl[:], OMEGA_LANG, None, OP.mult)
                    nc.vector.reciprocal(rg[:], pog[:, 128:129])
                    nc.vector.tensor_scalar(rg[:], rg[:], OMEGA_GRAV, None, OP.mult)
                    ob = outp.tile([128, 128], F32, tag="ob")
                    ob2 = outp.tile([128, 128], F32, tag="ob2")
                    nc.vector.tensor_scalar(ob[:], pol[:, 0:128], rl[:], None, OP.mult)
                    nc.vector.scalar_tensor_tensor(ob2[:], pog[:, 0:128], rg[:], ob[:],
                                                   OP.mult, OP.add)
                    r0 = pos * 512 + j * 128
                    nc.sync.dma_start(out_d[r0:r0 + 128, :], ob2[:])

    nc.finalize()
    return nc


def _host_inputs(x, Wq, Wk, Wv, L_grav):
    """Build the 8 per-core input maps."""
    f16 = np.float16
    x = np.asarray(x, np.float32)
    Wq = np.asarray(Wq, np.float32)
    Wk = np.asarray(Wk, np.float32)
    Wv = np.asarray(Wv, np.float32)
    L = np.asarray(L_grav, np.float32)
    Wy = Wk @ L

    def warr(w):  # [1024,128] -> [128, 8*128] chunk-major for lhsT slices
        return np.ascontiguousarray(
            w.reshape(8, 128, 128).transpose(1, 0, 2).reshape(128, 8 * 128)
        ).astype(f16)

    wqa, wka, wva, wya = warr(Wq), warr(Wk), warr(Wv), warr(Wy)
    lga = L.astype(f16)

    iota = np.ascontiguousarray(
        np.broadcast_to(np.arange(512, dtype=np.float32), (128, 512))).astype(f16)

    def half_data(h):
        """thr [128,24] (mask = iota >= thr), lbias/gbias [128,24] for half h."""
        p = np.arange(128, dtype=np.float32)
        th = np.empty((128, 24), np.float32)
        lb = np.zeros(24, np.float32)
        gb = np.zeros(24, np.float32)
        for pos in range(2):
            nch = NCH[pos]
            slot0 = 0 if pos == 0 else NCH[0]
            gs = ((0, 12) if h == 0 else (4, 8))[pos]
            for kb in range(nch):
                slot = slot0 + kb
                m = kb - gs  # chunk index relative to group start
                if m < 0:
                    th[:, slot] = -1e9  # fully valid
                elif m < 4:
                    th[:, slot] = m * 128 + p  # causal band (group-local cols)
                else:
                    th[:, slot] = 1e9  # fully invalid (also bias-gated)
                    lb[slot] = NEG
                    gb[slot] = NEG
        lbf = np.broadcast_to(lb, (128, 24)).astype(np.float32).copy()
        gbf = np.broadcast_to(gb, (128, 24)).astype(np.float32).copy()
        return th, lbf, gbf

    halves = [half_data(0), half_data(1)]
    in_maps = []
    for core in range(8):
        b, h = core // 2, core % 2
        xTb = np.ascontiguousarray(x[b].T).astype(f16)  # [1024, 2048]
        if h == 0:
            xq = np.concatenate([xTb[:, 0:512], xTb[:, 1536:2048]], axis=1)
        else:
            xq = np.ascontiguousarray(xTb[:, 512:1536])
        th, lbf, gbf = halves[h]
        in_maps.append({
            "xT": xTb, "xqT": xq,
            "wq": wqa, "wk": wka, "wv": wva, "wy": wya, "lg": lga,
            "iota": iota, "thr": th, "lbias": lbf, "gbias": gbf,
        })
    return in_maps


def kernel(x, Wq, Wk, Wv, L_grav):
    import concourse.bass_utils as bass_utils

    if "nc" not in _CACHE:
        _CACHE["nc"] = _build_nc()
    nc = _CACHE["nc"]
    in_maps = _host_inputs(x, Wq, Wk, Wv, L_grav)

    trace = bool(os.environ.get("BASS_KERNEL_TRACE"))
    if trace:
        bass_utils.upload_artifacts = lambda tmpdir: f"file://{tmpdir}"
    res = bass_utils.run_bass_kernel_spmd(nc, in_maps, list(range(8)), trace=trace)
    if trace:
        _CACHE["exec_time_ns"] = res.exec_time_ns
        _CACHE["mean_exec_time_ns"] = res.mean_exec_time_ns

    out = np.empty((B, N, D_HEAD), np.float32)
    for core in range(8):
        b, h = core // 2, core % 2
        r = res.results[core]["out"]
        if h == 0:
            out[b, 0:512] = r[0:512]
            out[b, 1536:2048] = r[512:1024]
        else:
            out[b, 512:1024] = r[0:512]
            out[b, 1024:1536] = r[512:1024]
    return out

